# revision 16
# baseline (speedup 1.0000x reference)
"""Trainium2 Bass kernel for nn_CCM: per-pixel complex 3x3 conv mask.

Math (per batch element b, sharded 1 batch element per NeuronCore):
  y[t,f] = sum_{c=0..26} m[c,t,f] * (w_{k(c)} * X)[t+i(c)-2, f+j(c)-1]
where c = 9*k + 3*i + j, w_k = v[0,k] + 1j*v[1,k] (cube roots of unity),
X = xr + 1j*xi, zero padded (causal in t: 2 top; symmetric in f: 1,1).

Design:
  - m streamed HBM->SBUF as fp16 via gpsimd SWDGE cast-DMA (free convert,
    spreads over all 16 DMA engines), deep prefetch ring
  - DVE computes only the 54 products per core in fp16 (2x_1P mode)
  - TensorE accumulates products into PSUM via fp16 identity matmuls
    (start/stop accumulation groups), 4x512 cols per component
  - last 8 cols (tau=7, f>=249) accumulated on gpsimd in fp16 (PSUM is 16
    fp32/partition too small for 2x2056); gpsimd also paces the cast-DMAs
  - U planes (w_k * X) in fp16 with even/odd alignment copies so every
    tap slice is 4B-aligned (keeps the DVE 2x mode)
  - taps reordered k=0 first so products start as soon as the x planes
    land; U1/U2 plane prep happens in the DVE stream between k=0 and k=1
  - x-prep: cast-DMA to fp16 natural layout, fp16 PE transposes for slots
    0..7 (4 slots batched per PSUM tile), slots 8/9 via SBUF halo DMA
  - output: PSUM evac (scalar) to fp16 staging, fp16 PE transposes, DVE
    strided copies into [f, (t c)] staging, gpsimd cast-DMA fp16->f32 out

Layout: t = 8*p + tau, partitions p in [0,125), (tau, f) in the free dim.
U planes are [128, 10 tau-slots, 260 f-cols] (slots tau=-2..7, f=-1..259).
"""

import sys
import numpy as np

sys.path.insert(0, "/opt/trn_rl_repo")

B = 8
C = 27
T = 1000
F = 257
TP = 125          # partitions with real data
TAU = 8           # t = 8*p + tau
NS = 10           # tau slots in U planes: tau in [-2, 8)
FPX = 260         # padded f width (even, for fp16 alignment): f in [-1, 259)
NFREE = TAU * F   # 2056
MAIN = 2048       # PE-accumulated cols (4 psum banks x 512)
TAILW = NFREE - MAIN  # 8
TPAD = 1024       # padded t for the output staging
TTP = 1008        # padded tt for x staging (full 126-partition transposes)
SQ3H = float(np.sqrt(3.0) / 2.0)
# overlapping 128-wide f chunks covering f in [0,257)
FCH = [0, 128, 129]
# tap order: k=0 even-phase, k=0 odd, k=1 even, k=2 even, k=1 odd, k=2 odd
# (df == 0 taps use the odd-phase planes)
ORDER = [0, 2, 3, 5, 6, 8, 1, 4, 7,
         9, 11, 12, 14, 15, 17, 18, 20, 21, 23, 24, 26,
         10, 13, 16, 19, 22, 25]

_CACHE = {}


def _emit(ctx, tc, m_ap, x_ap, id16_ap, y_ap):
    import concourse.mybir as mybir

    nc = tc.nc
    f32 = mybir.dt.float32
    f16 = mybir.dt.float16

    const = ctx.enter_context(tc.tile_pool(name="const", bufs=1))
    planes = ctx.enter_context(tc.tile_pool(name="planes", bufs=1))

    ident16 = const.tile([128, 128], f16, tag="ident16")
    nc.sync.dma_start(ident16[:], id16_ap)

    # ---- U planes (fp16): xq_e = even-phase, xq_o = odd-phase (shift left 1)
    xq_e, xq_o = [], []
    for q in range(2):
        pe_ = planes.tile([128, NS, FPX], f16, tag=f"xqe{q}", name="xqe")
        nc.vector.memset(pe_[:], 0.0)
        xq_e.append(pe_)
        po_ = planes.tile([128, NS, FPX], f16, tag=f"xqo{q}", name="xqo")
        xq_o.append(po_)

    # fp16 staging of the accumulated result; rows 125..127 must stay zero
    acc16_r = planes.tile([128, NFREE], f16, tag="acc16r")
    acc16_i = planes.tile([128, NFREE], f16, tag="acc16i")
    nc.vector.memset(acc16_r[:], 0.0)
    nc.vector.memset(acc16_i[:], 0.0)

    # ---- x natural fp16 [f, (tt, comp)] (tt = t + 2) via gpsimd cast-DMA,
    # then fp16 PE transposes into xq_e slots 0..7 (4 slots per PSUM tile,
    # copies split between DVE and scalar), slots 8/9 via SBUF halo DMA.
    with tc.tile_pool(name="xnp", bufs=1) as xnpool, \
         tc.tile_pool(name="psumx", bufs=3, space="PSUM") as psumx:
        xns = []
        for ci, f0 in enumerate(FCH):
            xn = xnpool.tile([128, TTP * 2], f16, tag=f"xn{ci}", name="xn")
            nc.gpsimd.memset(xn[:, 0:4], 0.0)
            nc.gpsimd.memset(xn[:, (T + 2) * 2:], 0.0)
            nc.gpsimd.dma_start(
                xn[:, 4:(T + 2) * 2],
                x_ap[f0:f0 + 128].rearrange("f t c -> f (t c)"),
            )
            xns.append(xn)
        ncp = 0
        for q in range(2):
            for tsg in range(2):          # slot groups 0..3, 4..7
                for ci, f0 in enumerate(FCH):
                    pt = psumx.tile([126, 4, 128], f16, tag="tp")
                    xn3 = xns[ci].rearrange("f (t c) -> f t c", c=2)
                    for k in range(4):
                        ts = 4 * tsg + k
                        nc.tensor.transpose(
                            pt[0:126, k, :],
                            xn3[:, ts:ts + TAU * 125 + 1:TAU, q],
                            ident16[0:128, 0:128],
                        )
                    dst = xq_e[q][0:126, 4 * tsg:4 * tsg + 4, 1 + f0:129 + f0]
                    if ncp % 2 == 0:
                        nc.vector.tensor_copy(dst, pt[0:126, :, :])
                    else:
                        nc.scalar.copy(dst, pt[0:126, :, :])
                    ncp += 1
        # halo: slots 8,9 of partition p = slots 0,1 of partition p+1
        for q in range(2):
            nc.sync.dma_start(
                xq_e[q][0:TP, 8:10, :], xq_e[q][1:TP + 1, 0:2, :]
            )

    NSF = NS * FPX
    # odd copies of X planes (scalar engine): xq_o[j] = xq_e[j+1]
    for q in range(2):
        ef = xq_e[q].rearrange("p a b -> p (a b)")
        of = xq_o[q].rearrange("p a b -> p (a b)")
        nc.scalar.copy(of[0:TP, 0:NSF - 1], ef[0:TP, 1:NSF])

    # U1/U2 plane tiles (even + odd phase)
    t1 = planes.tile([128, NS, FPX], f16, tag="t1")
    t2 = planes.tile([128, NS, FPX], f16, tag="t2")
    ta = planes.tile([128, NS, FPX], f16, tag="ta")
    tb = planes.tile([128, NS, FPX], f16, tag="tb")
    ur1e = planes.tile([128, NS, FPX], f16, tag="ur1e")
    ui1e = planes.tile([128, NS, FPX], f16, tag="ui1e")
    ur2e = planes.tile([128, NS, FPX], f16, tag="ur2e")
    ui2e = planes.tile([128, NS, FPX], f16, tag="ui2e")
    ur1o = planes.tile([128, NS, FPX], f16, tag="ur1o")
    ui1o = planes.tile([128, NS, FPX], f16, tag="ui1o")
    ur2o = planes.tile([128, NS, FPX], f16, tag="ur2o")
    ui2o = planes.tile([128, NS, FPX], f16, tag="ui2o")

    def emit_u_even():
        # U_k = w_k * (xr + i xi), w_1/2 = -0.5 +- i*s : DVE fp16.
        # tensor_scalar runs 4x and tensor_tensor 2x in fp16.
        nc.vector.tensor_scalar_mul(t1[0:TP], xq_e[1][0:TP], SQ3H)   # s*xi
        nc.vector.tensor_scalar_mul(t2[0:TP], xq_e[0][0:TP], SQ3H)   # s*xr
        nc.vector.tensor_scalar_mul(ta[0:TP], xq_e[0][0:TP], -0.5)
        nc.vector.tensor_scalar_mul(tb[0:TP], xq_e[1][0:TP], -0.5)
        nc.vector.tensor_sub(ur1e[0:TP], ta[0:TP], t1[0:TP])
        nc.vector.tensor_add(ui1e[0:TP], tb[0:TP], t2[0:TP])
        nc.vector.tensor_add(ur2e[0:TP], ta[0:TP], t1[0:TP])
        nc.vector.tensor_sub(ui2e[0:TP], tb[0:TP], t2[0:TP])
        # odd copies of U1/U2 on the scalar engine
        for src, dst in ((ur1e, ur1o), (ui1e, ui1o),
                         (ur2e, ur2o), (ui2e, ui2o)):
            sf = src.rearrange("p a b -> p (a b)")
            df_ = dst.rearrange("p a b -> p (a b)")
            nc.scalar.copy(df_[0:TP, 0:NSF - 1], sf[0:TP, 1:NSF])

    Ue = [(xq_e[0], xq_e[1]), (ur1e, ui1e), (ur2e, ui2e)]
    Uo = [(xq_o[0], xq_o[1]), (ur1o, ui1o), (ur2o, ui2o)]

    # fp16 accumulators for the 8-col tails (tau=7, f in [249,257))
    tail_r = planes.tile([TP, TAILW], f16, tag="tailr")
    tail_i = planes.tile([TP, TAILW], f16, tag="taili")
    nc.gpsimd.memset(tail_r[:], 0.0)
    nc.gpsimd.memset(tail_i[:], 0.0)

    # ---- tap loop: pr = m_c * U_sel (DVE fp16 2x); PE accumulates into PSUM
    # via identity matmuls. Tail cols accumulate on gpsimd, which also drives
    # the cast-DMAs (gen stays AHEAD taps in front of the tail reads).
    mpool = ctx.enter_context(tc.tile_pool(name="mtiles", bufs=14))
    prpool = ctx.enter_context(tc.tile_pool(name="prod", bufs=6))
    with tc.tile_pool(name="psacc", bufs=1, space="PSUM") as psacc:
        accR = [psacc.tile([TP, 512], f32, tag=f"aR{j}", name=f"aR{j}")
                for j in range(4)]
        accI = [psacc.tile([TP, 512], f32, tag=f"aI{j}", name=f"aI{j}")
                for j in range(4)]

        def issue_m(c):
            mt = mpool.tile([TP, NFREE], f16, tag="mt", name="mt")
            nc.gpsimd.dma_start(mt[:], m_ap[c].rearrange("(p t) f -> p (t f)", p=TP))
            return mt

        def accum(psum_chunks, pr, start, stop):
            for jc in range(4):
                nc.tensor.matmul(
                    psum_chunks[jc][:, :], ident16[0:TP, 0:TP],
                    pr[:, 512 * jc:512 * (jc + 1)],
                    start=start, stop=stop,
                )

        AHEAD = 3
        mts = {}
        for idx in range(AHEAD):
            mts[ORDER[idx]] = issue_m(ORDER[idx])
        for idx, c in enumerate(ORDER):
            if idx == 9:
                emit_u_even()   # U1/U2 prep sits between k=0 and k=1 taps
            kk, n = divmod(c, 9)
            i, j = divmod(n, 3)
            dt, df = i - 2, j - 1
            if df == 0:
                ur, ui = Uo[kk]
                fc = 0
            else:
                ur, ui = Ue[kk]
                fc = df + 1
            urs = ur[0:TP, dt + 2:dt + 2 + TAU, fc:fc + F]
            uis = ui[0:TP, dt + 2:dt + 2 + TAU, fc:fc + F]
            m3 = mts[c].rearrange("p (t f) -> p t f", f=F)
            start, stop = (idx == 0), (idx == C - 1)

            pr = prpool.tile([TP, NFREE], f16, tag="pr")
            pr3 = pr.rearrange("p (t f) -> p t f", f=F)
            nc.vector.tensor_mul(pr3[:], m3[:], urs)
            accum(accR, pr, start, stop)

            pi = prpool.tile([TP, NFREE], f16, tag="pr")
            pi3 = pi.rearrange("p (t f) -> p t f", f=F)
            nc.vector.tensor_mul(pi3[:], m3[:], uis)
            accum(accI, pi, start, stop)

            # pool engine: prefetch m tile, then tail-adds for this tap
            if idx + AHEAD < C:
                cn = ORDER[idx + AHEAD]
                mts[cn] = issue_m(cn)
            nc.gpsimd.tensor_add(tail_r[:], tail_r[:], pr[:, MAIN:NFREE])
            nc.gpsimd.tensor_add(tail_i[:], tail_i[:], pi[:, MAIN:NFREE])

        # evacuate PSUM (fp32) -> SBUF fp16 staging
        for jc in range(4):
            nc.scalar.copy(acc16_r[0:TP, 512 * jc:512 * (jc + 1)], accR[jc][:, :])
            nc.scalar.copy(acc16_i[0:TP, 512 * jc:512 * (jc + 1)], accI[jc][:, :])
        nc.scalar.copy(acc16_r[0:TP, MAIN:NFREE], tail_r[:])
        nc.scalar.copy(acc16_i[0:TP, MAIN:NFREE], tail_i[:])

    # ---- output: fp16 PE transposes (4 slots batched per PSUM tile), DVE
    # strided copies into yo16 [f, (t c)] fp16 staging, gpsimd cast-DMA out.
    # FCH chunks overlap at f in [129,256]; the f0=128 chunk only writes the
    # single HBM row f=128 to avoid double-writing.
    acc3 = [acc16_r.rearrange("p (t f) -> p t f", f=F),
            acc16_i.rearrange("p (t f) -> p t f", f=F)]
    yopool = ctx.enter_context(tc.tile_pool(name="yop", bufs=1))
    with tc.tile_pool(name="psumo", bufs=3, space="PSUM") as psumo:
        for ci, f0 in enumerate(FCH):
            yo = yopool.tile([128, TPAD, 2], f16, tag=f"yo{ci}", name="yo")
            # view [f, ts, p, c] of the (t = 8p + ts, c) free layout
            yov = yo.rearrange("f (p ts) c -> f ts p c", ts=TAU)
            for comp in range(2):
                for tsg in range(2):
                    po = psumo.tile([128, 4, 128], f16, tag="po")
                    for k in range(4):
                        ts = 4 * tsg + k
                        nc.tensor.transpose(
                            po[:, k, :],
                            acc3[comp][:, ts, f0:f0 + 128],
                            ident16[0:128, 0:128],
                        )
                    nc.vector.tensor_copy(
                        yov[:, 4 * tsg:4 * tsg + 4, :, comp], po[:, :, :]
                    )
            if ci == 1:
                nc.gpsimd.dma_start(
                    y_ap[128:129].rearrange("f t c -> f (t c)"),
                    yo.rearrange("f t c -> f (t c)")[0:1, 0:T * 2],
                )
            else:
                nc.gpsimd.dma_start(
                    y_ap[f0:f0 + 128].rearrange("f t c -> f (t c)"),
                    yo.rearrange("f t c -> f (t c)")[:, 0:T * 2],
                )


def _build():
    if "nc" in _CACHE:
        return _CACHE["nc"]
    from contextlib import ExitStack
    from concourse import bacc, mybir
    import concourse.tile as tile

    f32 = mybir.dt.float32
    f16 = mybir.dt.float16
    nc = bacc.Bacc("TRN2", target_bir_lowering=False, debug=False, num_devices=B)
    m_d = nc.dram_tensor("m", (C, T, F), f32, kind="ExternalInput")
    x_d = nc.dram_tensor("x", (F, T, 2), f32, kind="ExternalInput")
    id16_d = nc.dram_tensor("ident16", (128, 128), f16, kind="ExternalInput")
    y_d = nc.dram_tensor("y", (F, T, 2), f32, kind="ExternalOutput")

    with tile.TileContext(nc) as tc:
        with ExitStack() as ctx:
            _emit(ctx, tc, m_d.ap(), x_d.ap(), id16_d.ap(), y_d.ap())
    nc.compile()
    _CACHE["nc"] = nc
    return nc


def _in_maps(m, x):
    ident16 = np.eye(128, dtype=np.float16)
    return [
        {"m": np.ascontiguousarray(m[b]), "x": np.ascontiguousarray(x[b]),
         "ident16": ident16}
        for b in range(B)
    ]


def kernel(m, x, v, _trace=False):
    from concourse import bass_utils

    m = np.asarray(m, dtype=np.float32)
    x = np.asarray(x, dtype=np.float32)
    nc = _build()
    res = bass_utils.run_bass_kernel_spmd(
        nc, _in_maps(m, x), core_ids=list(range(B)), trace=_trace
    )
    kernel.last_results = res
    y = np.stack([res.results[b]["y"] for b in range(B)], axis=0)
    return y


# revision 20
# speedup vs baseline: 1.0353x; 1.0353x over previous
"""Trainium2 Bass kernel for nn_CCM: per-pixel complex 3x3 conv mask.

Math (per batch element b, sharded 1 batch element per NeuronCore):
  y[t,f] = sum_{c=0..26} m[c,t,f] * (w_{k(c)} * X)[t+i(c)-2, f+j(c)-1]
where c = 9*k + 3*i + j, w_k = v[0,k] + 1j*v[1,k] (cube roots of unity),
X = xr + 1j*xi, zero padded (causal in t: 2 top; symmetric in f: 1,1).

Design:
  - m streamed HBM->SBUF as fp16 via gpsimd SWDGE cast-DMA (free convert,
    spreads over all 16 DMA engines); m/product pools are allocated below
    the x staging so the stream never waits on prep-phase readers
  - DVE computes only the 54 products per core in fp16 (2x_1P mode)
  - TensorE accumulates products into PSUM via fp16 identity matmuls
    (start/stop accumulation groups), 4x512 cols per component
  - component-phased tap loop: R(14) I(14) R(13) I(13); each m tile is
    read twice while resident. The real-part output stage (evac + fp16
    PE transposes + copies) hides under the last imag product phase
  - last 8 cols (tau=7, f>=249) accumulated on gpsimd in fp16 (PSUM is
    16 fp32/partition too small for 2x2056); gpsimd also paces the DMAs
  - U planes (w_k * X) in fp16 with even/odd alignment copies so every
    tap slice is 4B-aligned; taps reordered k=0 first so products start
    as soon as the x planes land
  - output staged as fp16 [f, (t c)], gpsimd cast-DMA fp16->f32 to HBM

Layout: t = 8*p + tau, partitions p in [0,125), (tau, f) in the free dim.
U planes are [128, 10 tau-slots, 260 f-cols] (slots tau=-2..7, f=-1..259).
"""

import sys
import numpy as np

sys.path.insert(0, "/opt/trn_rl_repo")

B = 8
C = 27
T = 1000
F = 257
TP = 125          # partitions with real data
TAU = 8           # t = 8*p + tau
NS = 10           # tau slots in U planes: tau in [-2, 8)
FPX = 260         # padded f width (even, for fp16 alignment): f in [-1, 259)
NFREE = TAU * F   # 2056
MAIN = 2048       # PE-accumulated cols (4 psum banks x 512)
TAILW = NFREE - MAIN  # 8
TPAD = 1024       # padded t for the output staging
TTP = 1008        # padded tt for x staging (full 126-partition transposes)
SQ3H = float(np.sqrt(3.0) / 2.0)
# overlapping 128-wide f chunks covering f in [0,257)
FCH = [0, 128, 129]
# tap order: k=0 even-phase, k=0 odd, k=1 even, k=2 even, k=1 odd, k=2 odd
# (df == 0 taps use the odd-phase planes)
ORDER = [0, 2, 3, 5, 6, 8, 1, 4, 7,
         9, 11, 12, 14, 15, 17, 18, 20, 21, 23, 24, 26,
         10, 13, 16, 19, 22, 25]
NP1 = 14
P1, P2 = ORDER[:NP1], ORDER[NP1:]

_CACHE = {}


def _emit(ctx, tc, m_ap, x_ap, id16_ap, y_ap):
    import concourse.mybir as mybir

    nc = tc.nc
    f32 = mybir.dt.float32
    f16 = mybir.dt.float16

    const = ctx.enter_context(tc.tile_pool(name="const", bufs=1))
    planes = ctx.enter_context(tc.tile_pool(name="planes", bufs=1))
    # m/product pools open BEFORE the x staging pool so their SBUF space
    # is not a WAR-reuse of prep-phase tiles (that would stall the stream)
    mpool = ctx.enter_context(tc.tile_pool(name="mtiles", bufs=14))
    prpool = ctx.enter_context(tc.tile_pool(name="prod", bufs=4))

    ident16 = const.tile([128, 128], f16, tag="ident16")
    nc.sync.dma_start(ident16[:], id16_ap)

    # ---- U planes (fp16): xq_e = even-phase, xq_o = odd-phase (shift left 1)
    xq_e, xq_o = [], []
    for q in range(2):
        pe_ = planes.tile([128, NS, FPX], f16, tag=f"xqe{q}", name="xqe")
        nc.vector.memset(pe_[:], 0.0)
        xq_e.append(pe_)
        po_ = planes.tile([128, NS, FPX], f16, tag=f"xqo{q}", name="xqo")
        xq_o.append(po_)

    # fp16 staging of the accumulated result; rows 125..127 must stay zero
    acc16_r = planes.tile([128, NFREE], f16, tag="acc16r")
    acc16_i = planes.tile([128, NFREE], f16, tag="acc16i")

    # ---- x natural fp16 [f, (tt, comp)] (tt = t + 2) via gpsimd cast-DMA,
    # then fp16 PE transposes into xq_e slots 0..7 (4 slots per PSUM tile,
    # scalar copies), slots 8/9 via SBUF halo DMA.
    with tc.tile_pool(name="xnp", bufs=1) as xnpool, \
         tc.tile_pool(name="psumx", bufs=3, space="PSUM") as psumx:
        xns = []
        for ci, f0 in enumerate(FCH):
            xn = xnpool.tile([128, TTP * 2], f16, tag=f"xn{ci}", name="xn")
            nc.gpsimd.memset(xn[:, 0:4], 0.0)
            nc.gpsimd.memset(xn[:, (T + 2) * 2:], 0.0)
            nc.gpsimd.dma_start(
                xn[:, 4:(T + 2) * 2],
                x_ap[f0:f0 + 128].rearrange("f t c -> f (t c)"),
            )
            xns.append(xn)
        for q in range(2):
            for tsg in range(2):          # slot groups 0..3, 4..7
                for ci, f0 in enumerate(FCH):
                    pt = psumx.tile([126, 4, 128], f16, tag="tp")
                    xn3 = xns[ci].rearrange("f (t c) -> f t c", c=2)
                    for k in range(4):
                        ts = 4 * tsg + k
                        nc.tensor.transpose(
                            pt[0:126, k, :],
                            xn3[:, ts:ts + TAU * 125 + 1:TAU, q],
                            ident16[0:128, 0:128],
                        )
                    nc.scalar.copy(
                        xq_e[q][0:126, 4 * tsg:4 * tsg + 4, 1 + f0:129 + f0],
                        pt[0:126, :, :],
                    )
        # halo: slots 8,9 of partition p = slots 0,1 of partition p+1
        for q in range(2):
            nc.sync.dma_start(
                xq_e[q][0:TP, 8:10, :], xq_e[q][1:TP + 1, 0:2, :]
            )

    NSF = NS * FPX
    # odd copies of X planes (scalar engine): xq_o[j] = xq_e[j+1]
    for q in range(2):
        ef = xq_e[q].rearrange("p a b -> p (a b)")
        of = xq_o[q].rearrange("p a b -> p (a b)")
        nc.scalar.copy(of[0:TP, 0:NSF - 1], ef[0:TP, 1:NSF])

    # U1/U2 plane tiles (even + odd phase)
    t1 = planes.tile([128, NS, FPX], f16, tag="t1")
    t2 = planes.tile([128, NS, FPX], f16, tag="t2")
    ta = planes.tile([128, NS, FPX], f16, tag="ta")
    tb = planes.tile([128, NS, FPX], f16, tag="tb")
    ur1e = planes.tile([128, NS, FPX], f16, tag="ur1e")
    ui1e = planes.tile([128, NS, FPX], f16, tag="ui1e")
    ur2e = planes.tile([128, NS, FPX], f16, tag="ur2e")
    ui2e = planes.tile([128, NS, FPX], f16, tag="ui2e")
    ur1o = planes.tile([128, NS, FPX], f16, tag="ur1o")
    ui1o = planes.tile([128, NS, FPX], f16, tag="ui1o")
    ur2o = planes.tile([128, NS, FPX], f16, tag="ur2o")
    ui2o = planes.tile([128, NS, FPX], f16, tag="ui2o")

    def emit_u_even():
        # U_k = w_k * (xr + i xi), w_1/2 = -0.5 +- i*s : DVE fp16.
        nc.vector.tensor_scalar_mul(t1[0:TP], xq_e[1][0:TP], SQ3H)   # s*xi
        nc.vector.tensor_scalar_mul(t2[0:TP], xq_e[0][0:TP], SQ3H)   # s*xr
        nc.vector.tensor_scalar_mul(ta[0:TP], xq_e[0][0:TP], -0.5)
        nc.vector.tensor_scalar_mul(tb[0:TP], xq_e[1][0:TP], -0.5)
        nc.vector.tensor_sub(ur1e[0:TP], ta[0:TP], t1[0:TP])
        nc.vector.tensor_add(ui1e[0:TP], tb[0:TP], t2[0:TP])
        nc.vector.tensor_add(ur2e[0:TP], ta[0:TP], t1[0:TP])
        nc.vector.tensor_sub(ui2e[0:TP], tb[0:TP], t2[0:TP])
        # odd copies of U1/U2 on the scalar engine
        for src, dst in ((ur1e, ur1o), (ui1e, ui1o),
                         (ur2e, ur2o), (ui2e, ui2o)):
            sf = src.rearrange("p a b -> p (a b)")
            df_ = dst.rearrange("p a b -> p (a b)")
            nc.scalar.copy(df_[0:TP, 0:NSF - 1], sf[0:TP, 1:NSF])

    Ue = [(xq_e[0], xq_e[1]), (ur1e, ui1e), (ur2e, ui2e)]
    Uo = [(xq_o[0], xq_o[1]), (ur1o, ui1o), (ur2o, ui2o)]

    # fp16 accumulators for the 8-col tails (tau=7, f in [249,257))
    tail_r = planes.tile([TP, TAILW], f16, tag="tailr")
    tail_i = planes.tile([TP, TAILW], f16, tag="taili")
    nc.gpsimd.memset(tail_r[:], 0.0)
    nc.gpsimd.memset(tail_i[:], 0.0)

    def u_slice(c):
        kk, n = divmod(c, 9)
        i, j = divmod(n, 3)
        dt, df = i - 2, j - 1
        if df == 0:
            ur, ui = Uo[kk]
            fc = 0
        else:
            ur, ui = Ue[kk]
            fc = df + 1
        urs = ur[0:TP, dt + 2:dt + 2 + TAU, fc:fc + F]
        uis = ui[0:TP, dt + 2:dt + 2 + TAU, fc:fc + F]
        return urs, uis

    mts = {}

    def issue_m(c):
        mt = mpool.tile([TP, NFREE], f16, tag="mt", name="mt")
        nc.gpsimd.dma_start(mt[:], m_ap[c].rearrange("(p t) f -> p (t f)", p=TP))
        mts[c] = mt

    def product(c, which, acc_banks, start, stop, tail):
        urs, uis = u_slice(c)
        us = urs if which == 0 else uis
        m3 = mts[c].rearrange("p (t f) -> p t f", f=F)
        pr = prpool.tile([TP, NFREE], f16, tag="pr", name="pr")
        pr3 = pr.rearrange("p (t f) -> p t f", f=F)
        nc.vector.tensor_mul(pr3[:], m3[:], us)
        for jc in range(4):
            nc.tensor.matmul(
                acc_banks[jc][:, :], ident16[0:TP, 0:TP],
                pr[:, 512 * jc:512 * (jc + 1)],
                start=start, stop=stop,
            )
        return pr, tail

    # PSUM: separate 4-bank pools per component so the R pool can be
    # released (for the output-transpose pool) while I still accumulates.
    psI_cm = tc.tile_pool(name="psI", bufs=1, space="PSUM")
    psI = psI_cm.__enter__()
    psR_cm = tc.tile_pool(name="psR", bufs=1, space="PSUM")
    psR = psR_cm.__enter__()
    accR = [psR.tile([TP, 512], f32, tag=f"aR{j}", name=f"aR{j}")
            for j in range(4)]
    accI = [psI.tile([TP, 512], f32, tag=f"aI{j}", name=f"aI{j}")
            for j in range(4)]

    AHEAD = 3
    for c in P1[:AHEAD]:
        issue_m(c)

    # phase R1
    for j, c in enumerate(P1):
        if j == 9:
            emit_u_even()
        pr, _ = product(c, 0, accR, start=(j == 0), stop=False, tail=tail_r)
        if j + AHEAD < NP1:
            issue_m(P1[j + AHEAD])
        nc.gpsimd.tensor_add(tail_r[:], tail_r[:], pr[:, MAIN:NFREE])
    # phase I1 (re-reads the resident P1 tiles; prefetches P2)
    for j, c in enumerate(P1):
        pi, _ = product(c, 1, accI, start=(j == 0), stop=False, tail=tail_i)
        if j < len(P2):
            issue_m(P2[j])
        nc.gpsimd.tensor_add(tail_i[:], tail_i[:], pi[:, MAIN:NFREE])
    # phase R2 (closes the R accumulation)
    for j, c in enumerate(P2):
        pr, _ = product(c, 0, accR, start=False, stop=(j == len(P2) - 1),
                        tail=tail_r)
        nc.gpsimd.tensor_add(tail_r[:], tail_r[:], pr[:, MAIN:NFREE])

    # evac R: PSUM fp32 -> SBUF fp16 (scalar); memset zero rows first (DVE)
    nc.vector.memset(acc16_r[:], 0.0)
    for jc in range(4):
        nc.scalar.copy(acc16_r[0:TP, 512 * jc:512 * (jc + 1)], accR[jc][:, :])
    nc.scalar.copy(acc16_r[0:TP, MAIN:NFREE], tail_r[:])
    psR_cm.__exit__(None, None, None)

    # output staging pools
    acc3 = [acc16_r.rearrange("p (t f) -> p t f", f=F),
            acc16_i.rearrange("p (t f) -> p t f", f=F)]
    yopool = ctx.enter_context(tc.tile_pool(name="yop", bufs=1))
    yos = [yopool.tile([128, TPAD, 2], f16, tag=f"yo{ci}", name="yo")
           for ci in range(3)]
    psumo_cm = tc.tile_pool(name="psumo", bufs=3, space="PSUM")
    psumo = psumo_cm.__enter__()

    def out_group(comp, ci, tsg, on_dve):
        f0 = FCH[ci]
        yo = yos[ci]
        yov = yo.rearrange("f (p ts) c -> f ts p c", ts=TAU)
        po = psumo.tile([128, 4, 128], f16, tag="po", name="po")
        for k in range(4):
            ts = 4 * tsg + k
            nc.tensor.transpose(
                po[:, k, :], acc3[comp][:, ts, f0:f0 + 128],
                ident16[0:128, 0:128],
            )
        dst = yov[:, 4 * tsg:4 * tsg + 4, :, comp]
        if on_dve:
            nc.vector.tensor_copy(dst, po[:, :, :])
        else:
            nc.scalar.copy(dst, po[:, :, :])

    # phase I2 (closes I) with the R output stage interleaved on PE/scalar
    rgroups = [(ci, tsg) for ci in range(3) for tsg in range(2)]
    gi = 0
    for j, c in enumerate(P2):
        pi, _ = product(c, 1, accI, start=False, stop=(j == len(P2) - 1),
                        tail=tail_i)
        nc.gpsimd.tensor_add(tail_i[:], tail_i[:], pi[:, MAIN:NFREE])
        if j >= 1 and gi < 6:
            out_group(0, *rgroups[gi], on_dve=False)
            gi += 1
    while gi < 6:
        out_group(0, *rgroups[gi], on_dve=False)
        gi += 1

    # evac I + I output stage (exposed tail; copies on DVE, casts per chunk)
    nc.vector.memset(acc16_i[:], 0.0)
    for jc in range(4):
        nc.scalar.copy(acc16_i[0:TP, 512 * jc:512 * (jc + 1)], accI[jc][:, :])
    nc.scalar.copy(acc16_i[0:TP, MAIN:NFREE], tail_i[:])

    for ci in range(3):
        for tsg in range(2):
            out_group(1, ci, tsg, on_dve=True)
        yo = yos[ci]
        if ci == 1:
            nc.gpsimd.dma_start(
                y_ap[128:129].rearrange("f t c -> f (t c)"),
                yo.rearrange("f t c -> f (t c)")[0:1, 0:T * 2],
            )
        else:
            f0 = FCH[ci]
            nc.gpsimd.dma_start(
                y_ap[f0:f0 + 128].rearrange("f t c -> f (t c)"),
                yo.rearrange("f t c -> f (t c)")[:, 0:T * 2],
            )
    psumo_cm.__exit__(None, None, None)
    psI_cm.__exit__(None, None, None)


def _build():
    if "nc" in _CACHE:
        return _CACHE["nc"]
    from contextlib import ExitStack
    from concourse import bacc, mybir
    import concourse.tile as tile

    f32 = mybir.dt.float32
    f16 = mybir.dt.float16
    nc = bacc.Bacc("TRN2", target_bir_lowering=False, debug=False, num_devices=B)
    m_d = nc.dram_tensor("m", (C, T, F), f32, kind="ExternalInput")
    x_d = nc.dram_tensor("x", (F, T, 2), f32, kind="ExternalInput")
    id16_d = nc.dram_tensor("ident16", (128, 128), f16, kind="ExternalInput")
    y_d = nc.dram_tensor("y", (F, T, 2), f32, kind="ExternalOutput")

    with tile.TileContext(nc) as tc:
        with ExitStack() as ctx:
            _emit(ctx, tc, m_d.ap(), x_d.ap(), id16_d.ap(), y_d.ap())
    nc.compile()
    _CACHE["nc"] = nc
    return nc


def _in_maps(m, x):
    ident16 = np.eye(128, dtype=np.float16)
    return [
        {"m": np.ascontiguousarray(m[b]), "x": np.ascontiguousarray(x[b]),
         "ident16": ident16}
        for b in range(B)
    ]


def kernel(m, x, v, _trace=False):
    from concourse import bass_utils

    m = np.asarray(m, dtype=np.float32)
    x = np.asarray(x, dtype=np.float32)
    nc = _build()
    res = bass_utils.run_bass_kernel_spmd(
        nc, _in_maps(m, x), core_ids=list(range(B)), trace=_trace
    )
    kernel.last_results = res
    y = np.stack([res.results[b]["y"] for b in range(B)], axis=0)
    return y
